# revision 22
# baseline (speedup 1.0000x reference)
"""CapsuleLayer dynamic-routing kernel for 8 Trainium2 NeuronCores (v4).

Math (reference):
    u_hat[b,n,j,d] = sum_i W[n,j,d,i] * x[b,j,i]
    b = 0; for r in 0..2:
        c = softmax_n(b); s[b,n,d] = sum_j c*u_hat; v = squash_d(s)
        if r < 2: b += sum_d v*u_hat
    return v  [B, N, D]

Key identities:
  - logits_r = <V_r, u_hat> over d with V_r = v_0 + ... + v_{r-1}
    (logits accumulate additively, u_hat constant) -> no per-j state.
  - r0: c uniform = 1/N, so s0 = (1/N) sum_{j,i} x W — computed with
    x-stationary K=128 matmuls (full PE-row packing), accumulating
    [32b, 2048] directly in PSUM.  No transposes.

Sharding: J (2048) split 8 ways -> Jc=256/core; s AllReduce per
iteration (256 KiB); squash redundant on every core.

Free layout everywhere: f = q*512 + d*16 + nn  (n = 16q + nn).
This keeps every DVE op innermost-step-1 bf16 (2x mode), makes each
s-matmul quarter a contiguous 512-col slice, and each strip of the
s PSUM [32q+b, (d,nn)] compacts to [32, (q,d,nn)] with contiguous DMAs.

Per r>=1 group (4 j): K=64 block-diag-x matmuls -> u_ps; scalar-ACT
evac; tl = u*Vrep (DVE 2x); 5 halving TT-adds fold d -> logits;
exp+Z fused on Scalar (accum_out); c = e/Z; t2 = c*u split DVE/GpSimd;
4 col-tiled ones4 matmuls accumulate s strips in PSUM.
"""

import functools
import numpy as np

B, J, I = 32, 2048, 16
N, D = 64, 32
NCORES = 8
JC = J // NCORES          # 256 j per core
GRP = 4                   # j's per group
NG = JC // GRP            # 64 groups
OCT = 8                   # j's per r0 octet (K=128 stationary)
NO = JC // OCT            # 32 octets
DN = D * N                # 2048
ROUTINGS = 3
EPS = 1e-7


@functools.lru_cache(maxsize=1)
def _build():
    import concourse.mybir as mybir
    import concourse.bacc as bacc
    import concourse.tile as tile

    f32 = mybir.dt.float32
    bf16 = mybir.dt.bfloat16
    MUL = mybir.AluOpType.mult
    ADD = mybir.AluOpType.add
    AF = mybir.ActivationFunctionType
    AX = mybir.AxisListType.X

    nc = bacc.Bacc("TRN2", target_bir_lowering=False, debug=False,
                   num_devices=NCORES)

    w1_d = nc.dram_tensor("w1", [NG, GRP * I, DN], bf16, kind="ExternalInput")
    w2_d = nc.dram_tensor("w2", [NO, OCT * I, DN], bf16, kind="ExternalInput")
    xr_d = nc.dram_tensor("xr", [OCT * I, NO * B], bf16, kind="ExternalInput")
    xbd_d = nc.dram_tensor("xbd", [GRP * I, NG, 128], bf16, kind="ExternalInput")
    ones4_d = nc.dram_tensor("ones4", [GRP * B, B], bf16, kind="ExternalInput")
    v_d = nc.dram_tensor("v", [B, DN], f32, kind="ExternalOutput")

    with tile.TileContext(nc) as tc:
        with (
            tc.tile_pool(name="persist", bufs=1) as pp,
            tc.tile_pool(name="wstream", bufs=4) as wp,
            tc.tile_pool(name="work", bufs=4) as wk,
            tc.tile_pool(name="small", bufs=2) as sm,
            tc.tile_pool(name="ups", bufs=3, space="PSUM") as ups_pool,
            tc.tile_pool(name="sps", bufs=1, space="PSUM") as sps_pool,
            tc.tile_pool(name="dram", bufs=1, space="DRAM") as dr,
        ):
            xr = pp.tile([OCT * I, NO * B], bf16)
            nc.sync.dma_start(xr[:], xr_d[:])
            xbd = pp.tile([GRP * I, NG, 128], bf16)
            nc.sync.dma_start(xbd[:], xbd_d[:])
            ones4 = pp.tile([GRP * B, B], bf16)
            nc.sync.dma_start(ones4[:], ones4_d[:])

            VrepC = pp.tile([128, DN], bf16)     # cumulative V, replicated
            Vacc = pp.tile([128, 512], f32)      # cumulative V, strip layout
            zrow = pp.tile([1, 512], bf16)
            nc.vector.memset(zrow[:], 0.0)
            orow = pp.tile([1, 128], bf16)
            nc.vector.memset(orow[:], 0.0)
            eps_t = pp.tile([128, 1], f32)
            nc.vector.memset(eps_t[:], EPS)

            cc0_in = dr.tile([B, DN], bf16)
            cc0_out = dr.tile([B, DN], bf16)
            ccs_in = dr.tile([128, 512], bf16)
            ccs_out = dr.tile([128, 512], bf16)

            def squash_tail(s_sb, P, d_view, scl_bc, d_shape):
                """squash on s_sb [P, free]; returns v4 [P, free] f32.
                d_view: AP view [P, seg, d] with d strided for the reduce;
                scl_bc: fn scl -> broadcast AP matching s_sb's free shape."""
                sq = sm.tile(list(s_sb.shape), f32, name="sq", tag="it", bufs=2)
                nc.scalar.activation(sq[:], s_sb[:], AF.Square)
                ns2 = sm.tile([P, N // 4 if P == 128 else N], f32,
                              name="ns2", tag="pg", bufs=6)
                nc.vector.tensor_reduce(ns2[:], d_view(sq), axis=AX, op=ADD)
                onep = sm.tile(list(ns2.shape), f32, name="onep", tag="pg", bufs=6)
                nc.vector.tensor_scalar_add(onep[:], ns2[:], 1.0)
                rt = sm.tile(list(ns2.shape), f32, name="rt", tag="pg", bufs=6)
                nc.scalar.activation(rt[:], ns2[:], AF.Sqrt, bias=eps_t[:P])
                den = sm.tile(list(ns2.shape), f32, name="den", tag="pg", bufs=6)
                nc.vector.tensor_tensor(den[:], onep[:], rt[:], op=MUL)
                dinv = sm.tile(list(ns2.shape), f32, name="dinv", tag="pg", bufs=6)
                nc.vector.reciprocal(dinv[:], den[:])
                scl = sm.tile(list(ns2.shape), f32, name="scl", tag="pg", bufs=6)
                nc.vector.tensor_tensor(scl[:], ns2[:], dinv[:], op=MUL)
                v4 = sm.tile(list(s_sb.shape), f32, name="v4", tag="v4", bufs=2)
                bc = scl_bc(scl)
                nc.vector.tensor_tensor(d_shape(v4), d_shape(s_sb), bc, op=MUL)
                return v4

            def update_V(v4s, r):
                """Vacc (+)= v4s [128,512] strips; VrepC = replicate."""
                if r == 0:
                    nc.vector.tensor_copy(Vacc[:], v4s[:])
                else:
                    nc.vector.tensor_add(Vacc[:], Vacc[:], v4s[:])
                vb = sm.tile([128, 512], bf16, name="vb", tag="it", bufs=2)
                nc.vector.tensor_copy(vb[:], Vacc[:])
                for k in range(4):
                    for q in range(4):
                        nc.sync.dma_start(
                            VrepC[32 * k:32 * k + 32, 512 * q:512 * q + 512],
                            vb[32 * q:32 * q + 32, :])

            # ---------------- r0: x-stationary dense matmuls ----------------
            # start=True clears has_written for whole (rows x bank); clear
            # each bank once with a K=1 zero matmul, accumulate start=False.
            s0a = ups_pool.tile([B, DN // 2], f32, name="s0a", tag="u_ps")
            s0b = ups_pool.tile([B, DN // 2], f32, name="s0b", tag="u_ps")
            s0t = [s0a, s0a, s0b, s0b]
            for q in range(4):
                nc.tensor.matmul(s0t[q][:, 512 * (q % 2):512 * (q % 2) + 512],
                                 orow[:, :B], zrow[:],
                                 start=True, stop=False, skip_group_check=True)
            for o in range(NO):
                w2t = wp.tile([OCT * I, DN], bf16)
                nc.sync.dma_start(w2t[:], w2_d[o])
                for q in range(4):
                    nc.tensor.matmul(
                        s0t[q][:, 512 * (q % 2):512 * (q % 2) + 512],
                        xr[:, B * o:B * o + B],
                        w2t[:, 512 * q:512 * q + 512],
                        start=False, stop=(o == NO - 1),
                        skip_group_check=True,
                    )
            s_ar = sm.tile([B, DN], bf16, name="sar", tag="it", bufs=2)
            nc.scalar.activation(s_ar[:, 0:1024], s0a[:], AF.Copy,
                                 scale=1.0 / N)
            nc.scalar.activation(s_ar[:, 1024:2048], s0b[:], AF.Copy,
                                 scale=1.0 / N)
            nc.sync.dma_start(cc0_in[:], s_ar[:])
            nc.gpsimd.collective_compute(
                "AllReduce", ADD, replica_groups=[list(range(NCORES))],
                ins=[cc0_in[:].opt()], outs=[cc0_out[:].opt()],
            )
            ssb0 = sm.tile([B, DN], bf16, name="ssb0", tag="it", bufs=2)
            nc.sync.dma_start(ssb0[:], cc0_out[:])
            v4c = squash_tail(
                ssb0, B,
                lambda t: t[:].rearrange("b (q d nn) -> b q nn d",
                                         q=4, nn=16),
                lambda s: s[:].rearrange("b (q nn) -> b q nn", q=4)
                           [:, :, None, :].broadcast_to([B, 4, D, 16]),
                lambda t: t[:].rearrange("b (q d nn) -> b q d nn",
                                         q=4, nn=16))
            v4s0 = sm.tile([128, 512], f32, name="v4s0", tag="v4", bufs=2)
            for q in range(4):
                nc.sync.dma_start(v4s0[32 * q:32 * q + 32, :],
                                  v4c[:, 512 * q:512 * q + 512])
            update_V(v4s0, 0)

            # ---------------- r1, r2: routing sweeps ----------------
            for r in range(1, ROUTINGS):
                s_ps = sps_pool.tile([128, 512], f32, name="sps", tag="sp")
                pending_smm = []
                for g in range(NG):
                    w1t = wp.tile([GRP * I, DN], bf16)
                    nc.sync.dma_start(w1t[:], w1_d[g])

                    u_sb = wk.tile([128, 4, D, 16], bf16)   # [q, d, nn]
                    for h in range(2):
                        u_ps = ups_pool.tile([128, DN // 2], f32, name="u_ps", tag="u_ps")
                        for k in range(2):
                            nc.tensor.matmul(
                                u_ps[:, 512 * k:512 * k + 512], xbd[:, g, :],
                                w1t[:, 1024 * h + 512 * k:
                                    1024 * h + 512 * k + 512],
                                start=True, stop=True,
                            )
                        if pending_smm:
                            pending_smm.pop(0)()
                        nc.scalar.activation(
                            u_sb[:, 2 * h:2 * h + 2]
                            .rearrange("p a b c -> p (a b c)"),
                            u_ps[:], AF.Copy)

                    tl = wk.tile([128, 4, D, 16], bf16, name="tl", tag="tl",
                                 bufs=3)
                    nc.vector.tensor_tensor(
                        tl[:].rearrange("p a b c -> p (a b c)"),
                        u_sb[:].rearrange("p a b c -> p (a b c)"),
                        VrepC[:], op=MUL)
                    # fold d 32 -> 1 with 5 halving TT-adds
                    with nc.allow_low_precision("bf16 routing logits"):
                        t = tl
                        w = D
                        while w > 1:
                            w //= 2
                            nt = sm.tile([128, 4, w, 16], bf16,
                                         name=f"fd{w}", tag=f"fd{w}", bufs=4)
                            nc.vector.tensor_tensor(
                                nt[:], t[:, :, 0:w, :], t[:, :, w:2 * w, :],
                                op=ADD)
                            t = nt
                    logits = t                    # [128, 4, 1, 16] = [128, 64]

                    e_t = sm.tile([128, N], bf16, name="et", tag="pg", bufs=6)
                    zsum = sm.tile([128, 1], f32, name="zsum", tag="pg", bufs=6)
                    nc.scalar.activation(
                        e_t[:], logits[:].rearrange("p a b c -> p (a b c)"),
                        AF.Exp, accum_out=zsum[:])
                    t2 = wk.tile([128, 4, D, 16], bf16, name="t2", tag="t2")
                    nc.vector.tensor_tensor(
                        t2[:], u_sb[:],
                        e_t[:].rearrange("p (q nn) -> p q nn", q=4)
                        [:, :, None, :].broadcast_to([128, 4, D, 16]),
                        op=MUL)
                    # 1/Z folds into the strip-sum lhsT (per-partition scalar)
                    zrec = sm.tile([128, 1], f32, name="zrec", tag="pg", bufs=6)
                    nc.vector.reciprocal(zrec[:], zsum[:])
                    o4z = sm.tile([GRP * B, B], bf16, name="o4z", tag="pg",
                                  bufs=6)
                    nc.vector.tensor_scalar_mul(o4z[:], ones4[:], zrec[:])

                    def make_smm(t2=t2, o4z=o4z, g=g):
                        def emit():
                            for q in range(4):
                                nc.tensor.matmul(
                                    s_ps[32 * q:32 * q + 32, :],
                                    o4z[:],
                                    t2[:, q].rearrange("p a b -> p (a b)"),
                                    start=(g == 0), stop=(g == NG - 1),
                                    tile_position=(0, 32 * q),
                                    skip_group_check=True,
                                )
                        return emit
                    pending_smm.append(make_smm())
                while pending_smm:
                    pending_smm.pop(0)()

                # tail: AllReduce strips, squash in strips, compact v4
                s_ev = sm.tile([128, 512], bf16, name="sev", tag="sev", bufs=2)
                nc.vector.tensor_copy(s_ev[:], s_ps[:])
                nc.sync.dma_start(ccs_in[:], s_ev[:])
                nc.gpsimd.collective_compute(
                    "AllReduce", ADD, replica_groups=[list(range(NCORES))],
                    ins=[ccs_in[:].opt()], outs=[ccs_out[:].opt()],
                )
                ssb_s = sm.tile([128, 512], bf16, name="ssbs", tag="sev", bufs=2)
                nc.sync.dma_start(ssb_s[:], ccs_out[:])
                v4s = squash_tail(
                    ssb_s, 128,
                    lambda t: t[:].rearrange("p (d nn) -> p nn d", nn=16),
                    lambda s: s[:, None, :].broadcast_to([128, D, 16]),
                    lambda t: t[:].rearrange("p (d nn) -> p d nn", nn=16))
                if r < ROUTINGS - 1:
                    update_V(v4s, r)
                else:
                    v4c = sm.tile([B, DN], f32, name="v4c", tag="v4", bufs=2)
                    for q in range(4):
                        nc.sync.dma_start(v4c[:, 512 * q:512 * q + 512],
                                          v4s[32 * q:32 * q + 32, :])
                    nc.sync.dma_start(v_d[:], v4c[:])

    nc.compile()
    return nc


def prepare_in_maps(x: np.ndarray, W: np.ndarray):
    import ml_dtypes
    bf = ml_dtypes.bfloat16

    ones4 = np.tile(np.eye(B, dtype=np.float32), (GRP, 1)).astype(bf)

    in_maps = []
    for k in range(NCORES):
        jlo, jhi = k * JC, (k + 1) * JC
        Wc = W[:, jlo:jhi]                     # [N, 256, D, I]
        Wt = np.ascontiguousarray(Wc.transpose(1, 3, 2, 0))  # [j, i, d, n]
        # columns (q, d, nn): n = 16q + nn
        Wq = (Wt.reshape(JC, I, D, 4, 16).transpose(0, 1, 3, 2, 4)
              .reshape(JC, I, DN))
        w1 = Wq.reshape(NG, GRP * I, DN).astype(bf)
        w2 = Wq.reshape(NO, OCT * I, DN).astype(bf)
        xc = x[:, jlo:jhi]                     # [B, 256, I]
        xt_j = np.ascontiguousarray(xc.transpose(1, 2, 0))   # [j, i, b]
        xr = np.ascontiguousarray(
            xt_j.reshape(NO, OCT, I, B).transpose(1, 2, 0, 3)
        ).reshape(OCT * I, NO * B).astype(bf)
        xbd = np.zeros((GRP * I, NG, 128), dtype=np.float32)
        xv = xt_j.reshape(NG, GRP, I, B)       # [g, rr, i, b]
        for rr in range(GRP):
            xbd[16 * rr:16 * rr + 16, :, 32 * rr:32 * rr + 32] = \
                xv[:, rr].transpose(1, 0, 2)
        in_maps.append({
            "w1": np.ascontiguousarray(w1),
            "w2": np.ascontiguousarray(w2),
            "xr": xr,
            "xbd": xbd.astype(bf),
            "ones4": ones4,
        })
    return in_maps


def kernel(x: np.ndarray, W: np.ndarray) -> np.ndarray:
    from concourse.bass_utils import run_bass_kernel_spmd

    nc = _build()
    in_maps = prepare_in_maps(x, W)
    res = run_bass_kernel_spmd(nc, in_maps, list(range(NCORES)))
    v = np.asarray(res.results[0]["v"], dtype=np.float32)
    # v[b, q*512 + d*16 + nn] -> [B, N, D] with n = 16q + nn
    return np.ascontiguousarray(
        v.reshape(B, 4, D, 16).transpose(0, 1, 3, 2).reshape(B, N, D))


if __name__ == "__main__":
    rng = np.random.default_rng(0)
    x = rng.normal(size=(B, J, I)).astype(np.float32)
    W = rng.normal(size=(N, J, D, I)).astype(np.float32) * 0.05
    v = kernel(x, W)
    print(v.shape, v.dtype, np.abs(v).max())
